# revision 6
# baseline (speedup 1.0000x reference)
"""Trainium2 Bass kernel for nn_AttentionNetwork (B=16, S=H=1024).

reference:
    energy  = tanh(concat([ht bcast, enc], -1) @ W_attn.T + b_attn)   [B,S,H]
    att     = softmax(energy, axis=1)  (over the seq axis)
    context = einsum('bsk,bkh->bsh', att, enc)
    returns (context, att)   (the W_v projection output is dead code)

Strategy (v2):
  - Data-parallel over batch: 2 batches per NeuronCore x 8 cores (SPMD).
  - htE = ht @ W1.T + b_attn computed on HOST (tiny GEMM), shipped as an
    8KB f32 tensor; device only does the two big GEMMs per batch.
  - mm1 computes energy TRANSPOSED (energyT[h,s]) so softmax over s is a
    free-dim reduction. kt-OUTER loop order: each arriving 256KB encT
    chunk immediately unlocks work against all 8 output row-blocks
    (8 PSUM banks), so the PE never starves at the head (HAM stays warm).
  - softmax: tanh(+bias) PSUM->SBUF f32 on ACT, exp with accum_out sums,
    reciprocal on DVE, at = ex*rec as bf16 (DVE/GpSimd split).
  - att is written to DRAM in TRANSPOSED layout (attT[h,s], bf16) straight
    from the at tiles -- no PE transposes; the host transposes + upcasts.
  - mm2: ctx = matmul(lhsT=at, rhs=enc) natural [s,h]; PSUM evacuated as
    bf16 via scalar/vector copy halves; DMA out bf16, host upcasts.
  - PE program order: warm dummies, mm1(b0), mm1(b1), mm2(b0), mm2(b1) --
    no PE gaps; everything else rides in the shadow of the matmuls.
"""

import sys
import numpy as np

sys.path.insert(0, "/opt/trn_rl_repo")

import concourse.bass as bass
import concourse.mybir as mybir
import concourse.tile as tile
from concourse.bass_utils import run_bass_kernel_spmd
from concourse.masks import make_identity

F32 = mybir.dt.float32
BF = mybir.dt.bfloat16
AF = mybir.ActivationFunctionType

B, S, H = 16, 1024, 1024
NCORES = 8
BPC = B // NCORES  # batches per core
KT = 8             # 128-row contraction tiles
MT = 8             # output partition tiles
NH = 512           # matmul free-dim chunk (one PSUM bank fp32)
WARM_MM = 22       # [128,512] dummy matmuls bridging the DMA head


def _split_sync_waits(nc, maxw=1):
    """This walrus rejects instructions with more than one sync wait.
    Move excess on_wait entries onto InstNoOp on the same engine queue
    (executed in order ahead of the real instruction)."""
    ctr = 0
    for fn in nc.m.functions:
        for blk in fn.blocks:
            new = []
            for inst in blk.instructions:
                si = inst.sync_info
                if si is not None and si.on_wait and len(si.on_wait) > maxw:
                    waits = list(si.on_wait)
                    extra, keep = waits[:-maxw], waits[-maxw:]
                    for i in range(0, len(extra), maxw):
                        ctr += 1
                        nop = mybir.InstNoOp(
                            name=f"I-ws-{ctr}",
                            engine=inst.engine,
                            sync_info=mybir.SyncInfo(
                                on_wait=extra[i : i + maxw], on_update=[]
                            ),
                        )
                        nc.register_instruction(nop)
                        new.append(nop)
                    inst.sync_info = mybir.SyncInfo(
                        on_wait=keep, on_update=list(si.on_update)
                    )
                new.append(inst)
            blk.instructions[:] = new
    return ctr


def build():
    nc = bass.Bass()
    enc_d = nc.declare_dram_parameter("enc", [BPC, 128, KT * H], BF, isOutput=False)
    encT_d = nc.declare_dram_parameter("encT", [BPC, 128, KT * S], BF, isOutput=False)
    w2tk_d = nc.declare_dram_parameter("w2tk", [128, KT * H], BF, isOutput=False)
    htE_d = nc.declare_dram_parameter("htE", [128, MT * BPC], F32, isOutput=False)
    ctx_d = nc.declare_dram_parameter("ctx", [BPC, S, H], BF, isOutput=True)
    attT_d = nc.declare_dram_parameter("attT", [BPC, H, S], BF, isOutput=True)

    with tile.TileContext(nc) as tc:
        with (
            tc.tile_pool(name="wpool", bufs=1) as wpool,
            tc.tile_pool(name="etpool", bufs=2 * KT) as etpool,   # encT tiles
            tc.tile_pool(name="enpool", bufs=2 * KT) as enpool,   # enc tiles
            tc.tile_pool(name="epool", bufs=MT) as epool,         # eT f32
            tc.tile_pool(name="xpool", bufs=3) as xpool,          # ex bf16
            tc.tile_pool(name="apool", bufs=2 * KT) as apool,     # at bf16
            tc.tile_pool(name="spool", bufs=4) as spool,          # sums/rec
            tc.tile_pool(name="cstg", bufs=4) as cstg,            # ctx staging
            tc.tile_pool(name="ps", bufs=8, space="PSUM") as ps,  # all 8 banks
        ):
            # --- head DMAs, first-use order.
            # sync queue: encT(b0) per-kt chunks (mm1 b0 critical path)
            # scalar queue: w2tk per-kt chunks (co-critical), then enc b0/b1
            # gpsimd queue: htE (tiny), later att-out
            encT = [[None] * KT for _ in range(BPC)]
            enc = [[None] * KT for _ in range(BPC)]
            for kt in range(KT):
                et = etpool.tile([128, S], BF, tag="encT")
                nc.sync.dma_start(out=et[:], in_=encT_d[0, :, kt * S : (kt + 1) * S])
                encT[0][kt] = et
            w2tk = wpool.tile([128, KT * H], BF)
            for kt in range(KT):
                nc.scalar.dma_start(
                    out=w2tk[:, kt * H : (kt + 1) * H],
                    in_=w2tk_d[:, kt * H : (kt + 1) * H],
                )
            htE = wpool.tile([128, MT * BPC], F32)
            nc.gpsimd.dma_start(out=htE[:], in_=htE_d[:])
            for kt in range(KT):
                et = etpool.tile([128, S], BF, tag="encT")
                nc.sync.dma_start(out=et[:], in_=encT_d[1, :, kt * S : (kt + 1) * S])
                encT[1][kt] = et
            for b in range(BPC):
                for kt in range(KT):
                    e = enpool.tile([128, H], BF, tag="enc")
                    nc.scalar.dma_start(
                        out=e[:], in_=enc_d[b, :, kt * H : (kt + 1) * H]
                    )
                    enc[b][kt] = e

            # --- PE warmup: keep HAM warm while the first chunks stream in.
            dummy = wpool.tile([128, NH], BF)
            nc.vector.memset(dummy[:], 0.0)
            ident = wpool.tile([128, 128], BF)
            make_identity(nc, ident[:])
            warmp = ps.tile([128, NH], F32, tag="ps")
            for i in range(WARM_MM):
                nc.tensor.matmul(
                    warmp[:], ident[:], dummy[:], start=(i == 0), stop=(i == WARM_MM - 1)
                )
            # warm the ACT table (tanh/exp share one table set)
            warma = wpool.tile([128, 1], F32)
            nc.vector.memset(warma[:], 0.5)
            nc.scalar.activation(warma[:], warma[:], AF.Exp)

            def mm1(b):
                # energyT[h,s] block-row mt: sum_kt w2tk[kt]^T @ encT[kt]
                pss = [None] * MT
                eTs = [None] * MT
                for half in range(2):
                    cs = slice(half * NH, (half + 1) * NH)
                    for mt in range(MT):
                        if half == 0:
                            eTs[mt] = epool.tile(
                                [128, S], F32, tag="eT", name=f"eT_{b}_{mt}"
                            )
                    for kt in range(KT):
                        for mt in range(MT):
                            if kt == 0:
                                pss[mt] = ps.tile(
                                    [128, NH], F32, tag="ps", name=f"ps_{b}_{half}_{mt}"
                                )
                            nc.tensor.matmul(
                                pss[mt][:],
                                w2tk[:, kt * H + mt * 128 : kt * H + (mt + 1) * 128],
                                encT[b][kt][:, cs],
                                start=(kt == 0),
                                stop=(kt == KT - 1),
                            )
                            if kt == KT - 1:
                                # evacuate: eT = tanh(psum + htE[:,mt,b])
                                nc.scalar.activation(
                                    eTs[mt][:, cs],
                                    pss[mt][:],
                                    AF.Tanh,
                                    bias=htE[:, mt * BPC + b : mt * BPC + b + 1],
                                )
                return eTs

            def softmax(b, eTs):
                sums = spool.tile([128, MT], F32, tag="sums")
                rec = spool.tile([128, MT], F32, tag="rec")
                ats = []
                for mt in range(MT):
                    ex = xpool.tile([128, S], BF, tag="ex")
                    nc.scalar.activation(
                        ex[:], eTs[mt][:], AF.Exp, accum_out=sums[:, mt : mt + 1]
                    )
                    nc.vector.reciprocal(rec[:, mt : mt + 1], sums[:, mt : mt + 1])
                    at = apool.tile([128, S], BF, tag="at")
                    eng = nc.vector if mt % 2 == 0 else nc.gpsimd
                    eng.tensor_scalar_mul(at[:], ex[:], rec[:, mt : mt + 1])
                    nc.gpsimd.dma_start(
                        out=attT_d[b, mt * 128 : (mt + 1) * 128, :], in_=at[:]
                    )
                    ats.append(at)
                return ats

            def mm2(b, ats):
                for mt2 in range(MT):
                    p0 = ps.tile([128, NH], F32, tag="ps")
                    p1 = ps.tile([128, NH], F32, tag="ps")
                    for kt in range(KT):
                        lhs = ats[kt][:, mt2 * 128 : (mt2 + 1) * 128]
                        nc.tensor.matmul(
                            p0[:], lhs, enc[b][kt][:, :NH],
                            start=(kt == 0), stop=(kt == KT - 1),
                        )
                        nc.tensor.matmul(
                            p1[:], lhs, enc[b][kt][:, NH:],
                            start=(kt == 0), stop=(kt == KT - 1),
                        )
                    stg = cstg.tile([128, H], BF, tag="cstg")
                    nc.scalar.copy(out=stg[:, :NH], in_=p0[:])
                    nc.vector.tensor_copy(stg[:, NH:], p1[:])
                    nc.sync.dma_start(
                        out=ctx_d[b, mt2 * 128 : (mt2 + 1) * 128, :], in_=stg[:]
                    )

            eT0 = mm1(0)
            a0 = softmax(0, eT0)
            eT1 = mm1(1)
            mm2(0, a0)
            a1 = softmax(1, eT1)
            mm2(1, a1)

    _split_sync_waits(nc, 1)
    return nc


_NC_CACHE = {}


def _get_nc():
    if "nc" not in _NC_CACHE:
        _NC_CACHE["nc"] = build()
    return _NC_CACHE["nc"]


def _pack(m):
    # [1024, D] -> [128, 8*D] with 128-row tile kt at columns [kt*D,(kt+1)*D)
    d = m.shape[1]
    return np.ascontiguousarray(
        m.reshape(KT, 128, d).transpose(1, 0, 2).reshape(128, KT * d)
    )


def _make_in_maps(ht, enc, W_attn, b_attn):
    import ml_dtypes

    bf = ml_dtypes.bfloat16
    ht = np.asarray(ht, np.float32)
    enc = np.asarray(enc, np.float32)
    W = np.asarray(W_attn, np.float32)
    ba = np.asarray(b_attn, np.float32)

    # w2tk[p, kt*H + mt*128 + j] = W2T[kt*128+p, mt*128+j] (kt-major packing)
    w2tk_p = _pack(W[:, H:].T.copy()).astype(bf)
    # htE_full[b, h] = ht @ W1.T + b_attn  (computed on host, tiny)
    htE_full = (ht @ W[:, :H].T + ba).astype(np.float32)  # [B, H]

    in_maps = []
    for c in range(NCORES):
        bs = slice(BPC * c, BPC * (c + 1))
        enc_c = enc[bs]
        enc_p = np.stack([_pack(enc_c[i]) for i in range(BPC)]).astype(bf)
        encT_p = np.stack([_pack(enc_c[i].T.copy()) for i in range(BPC)]).astype(bf)
        # htE_col[p, mt*BPC + i] = htE_full[bs][i, mt*128 + p]
        htE_c = np.ascontiguousarray(
            htE_full[bs].reshape(BPC, MT, 128).transpose(2, 1, 0).reshape(128, MT * BPC)
        )
        in_maps.append(
            {"enc": enc_p, "encT": encT_p, "w2tk": w2tk_p, "htE": htE_c}
        )
    return in_maps


def _run(in_maps, trace=False):
    res = run_bass_kernel_spmd(
        _get_nc(), in_maps, core_ids=list(range(NCORES)), trace=trace
    )
    ctx = np.concatenate(
        [r["ctx"].astype(np.float32) for r in res.results], axis=0
    )
    att = np.concatenate(
        [r["attT"].transpose(0, 2, 1).astype(np.float32) for r in res.results],
        axis=0,
    )
    return (ctx, att), res


def kernel(ht, encoder_out, W_attn, b_attn, W_v=None, **_unused):
    out, _ = _run(_make_in_maps(ht, encoder_out, W_attn, b_attn), trace=False)
    return out


def kernel_traced(ht, encoder_out, W_attn, b_attn, W_v=None, **_unused):
    """Like kernel() but also returns the BassKernelResults with profile."""
    out, res = _run(_make_in_maps(ht, encoder_out, W_attn, b_attn), trace=True)
    return out, res


# revision 7
# speedup vs baseline: 1.5635x; 1.5635x over previous
"""Trainium2 Bass kernel for nn_AttentionNetwork (B=16, S=H=1024).

reference:
    energy  = tanh(concat([ht bcast, enc], -1) @ W_attn.T + b_attn)   [B,S,H]
    att     = softmax(energy, axis=1)  (over the seq axis)
    context = einsum('bsk,bkh->bsh', att, enc)
    returns (context, att)   (the W_v projection output is dead code)

Strategy (v3):
  - Data-parallel over batch: 2 batches per NeuronCore x 8 cores (SPMD).
  - htE = ht @ W1.T + b_attn computed on HOST (tiny GEMM), shipped as an
    8KB f32 tensor; device only does the two big GEMMs per batch.
  - mm1 computes energy TRANSPOSED (energyT[h,s]) so softmax over s is a
    free-dim reduction. kt-OUTER loop order: each arriving encT chunk
    immediately unlocks work against all 8 output row-blocks (8 PSUM
    banks), so the PE never starves at the head (HAM stays warm).
  - softmax: tanh(+bias) PSUM->SBUF f32 on ACT, exp with accum_out sums,
    reciprocal + at = ex*rec (bf16) on DVE (GpSimd tensor ops are slow
    and also wedge DVE's fast mode -- keep GpSimd to DMA triggers only).
  - att is written to DRAM TRANSPOSED (attT[h,s], bf16) straight from the
    at tiles -- no PE transposes; the host transposes + upcasts.
  - mm2: ctx = matmul(lhsT=at, rhs=enc) natural [s,h]; PSUM evacuated as
    bf16 via scalar/vector copy halves; DMA out bf16, host upcasts.
  - Engine queues: scalar = activations/copies ONLY (DMA triggers would
    head-of-line block the tanh chain); sync = encT/w2tk in + ctx out;
    gpsimd = htE/enc in + attT out; vector = recip/at-mult/ctx-copy.
  - PE program order: warm dummies, mm1(b0), mm1(b1), mm2(b0), mm2(b1)
    with no PE gaps; everything else rides in the matmul shadow.
"""

import sys
import numpy as np

sys.path.insert(0, "/opt/trn_rl_repo")

import concourse.bass as bass
import concourse.mybir as mybir
import concourse.tile as tile
from concourse.bass_utils import run_bass_kernel_spmd
from concourse.masks import make_identity

F32 = mybir.dt.float32
BF = mybir.dt.bfloat16
AF = mybir.ActivationFunctionType

B, S, H = 16, 1024, 1024
NCORES = 8
BPC = B // NCORES  # batches per core
KT = 8             # 128-row contraction tiles
MT = 8             # output partition tiles
NH = 512           # matmul free-dim chunk (one PSUM bank fp32)
WARM_MM = 22       # [128,512] dummy matmuls bridging the DMA head


def _split_sync_waits(nc, maxw=1):
    """This walrus rejects instructions with more than one sync wait.
    Move excess on_wait entries onto InstNoOp on the same engine queue
    (executed in order ahead of the real instruction)."""
    ctr = 0
    for fn in nc.m.functions:
        for blk in fn.blocks:
            new = []
            for inst in blk.instructions:
                si = inst.sync_info
                if si is not None and si.on_wait and len(si.on_wait) > maxw:
                    waits = list(si.on_wait)
                    extra, keep = waits[:-maxw], waits[-maxw:]
                    for i in range(0, len(extra), maxw):
                        ctr += 1
                        nop = mybir.InstNoOp(
                            name=f"I-ws-{ctr}",
                            engine=inst.engine,
                            sync_info=mybir.SyncInfo(
                                on_wait=extra[i : i + maxw], on_update=[]
                            ),
                        )
                        nc.register_instruction(nop)
                        new.append(nop)
                    inst.sync_info = mybir.SyncInfo(
                        on_wait=keep, on_update=list(si.on_update)
                    )
                new.append(inst)
            blk.instructions[:] = new
    return ctr


def build():
    nc = bass.Bass()
    enc_d = nc.declare_dram_parameter("enc", [BPC, 128, KT * H], BF, isOutput=False)
    encT_d = nc.declare_dram_parameter("encT", [BPC, 128, KT * S], BF, isOutput=False)
    w2tk_d = nc.declare_dram_parameter("w2tk", [128, KT * H], BF, isOutput=False)
    htE_d = nc.declare_dram_parameter("htE", [128, MT * BPC], F32, isOutput=False)
    ctx_d = nc.declare_dram_parameter("ctx", [BPC, S, H], BF, isOutput=True)
    attT_d = nc.declare_dram_parameter("attT", [BPC, H, S], BF, isOutput=True)

    with tile.TileContext(nc) as tc:
        with (
            tc.tile_pool(name="wpool", bufs=1) as wpool,
            tc.tile_pool(name="w2pool", bufs=KT // 2) as w2pool,  # w2tk 2-kt tiles
            tc.tile_pool(name="etpool", bufs=2 * KT // 2) as etpool,  # encT 2-kt
            tc.tile_pool(name="enpool", bufs=2 * KT // 4) as enpool,  # enc 4-kt
            tc.tile_pool(name="epool", bufs=MT) as epool,         # eT f32
            tc.tile_pool(name="xpool", bufs=3) as xpool,          # ex bf16
            tc.tile_pool(name="apool", bufs=2 * KT) as apool,     # at bf16
            tc.tile_pool(name="spool", bufs=4) as spool,          # sums/rec
            tc.tile_pool(name="cstg", bufs=4) as cstg,            # ctx staging
            tc.tile_pool(name="ps", bufs=8, space="PSUM") as ps,  # all 8 banks
        ):
            # --- ACT table warm first: nothing sits in front of it on the
            # scalar queue, so the 1.3us spline-table load happens at ~4us.
            warma = wpool.tile([128, 1], F32)
            nc.vector.memset(warma[:], 0.5)
            nc.scalar.activation(warma[:], warma[:], AF.Exp)

            # --- head DMAs, first-use order.
            # sync queue: encT(b0)/w2tk interleaved 2-kt chunks (mm1 b0
            # critical path), then encT(b1); later ctx out.
            # gpsimd queue: htE (tiny), enc b0/b1 4-kt chunks; later attT out.
            encT = [[None] * (KT // 2) for _ in range(BPC)]
            enc = [[None] * (KT // 4) for _ in range(BPC)]
            w2tk = [None] * (KT // 2)
            for kp in range(KT // 2):
                et = etpool.tile([128, 2 * S], BF, tag="encT", name=f"encT_0_{kp}")
                nc.sync.dma_start(
                    out=et[:], in_=encT_d[0, :, kp * 2 * S : (kp + 1) * 2 * S]
                )
                encT[0][kp] = et
                wt = w2pool.tile([128, 2 * H], BF, tag="w2tk", name=f"w2tk_{kp}")
                nc.sync.dma_start(
                    out=wt[:], in_=w2tk_d[:, kp * 2 * H : (kp + 1) * 2 * H]
                )
                w2tk[kp] = wt
            htE = wpool.tile([128, MT * BPC], F32)
            nc.gpsimd.dma_start(out=htE[:], in_=htE_d[:])
            for kp in range(KT // 2):
                et = etpool.tile([128, 2 * S], BF, tag="encT", name=f"encT_1_{kp}")
                nc.sync.dma_start(
                    out=et[:], in_=encT_d[1, :, kp * 2 * S : (kp + 1) * 2 * S]
                )
                encT[1][kp] = et
            for b in range(BPC):
                for kq in range(KT // 4):
                    e = enpool.tile([128, 4 * H], BF, tag="enc", name=f"enc_{b}_{kq}")
                    nc.gpsimd.dma_start(
                        out=e[:], in_=enc_d[b, :, kq * 4 * H : (kq + 1) * 4 * H]
                    )
                    enc[b][kq] = e

            # --- PE warmup: keep HAM warm while the first chunks stream in.
            dummy = wpool.tile([128, NH], BF)
            nc.vector.memset(dummy[:], 0.0)
            ident = wpool.tile([128, 128], BF)
            make_identity(nc, ident[:])
            warmp = ps.tile([128, NH], F32, tag="ps")
            for i in range(WARM_MM):
                nc.tensor.matmul(
                    warmp[:], ident[:], dummy[:], start=(i == 0), stop=(i == WARM_MM - 1)
                )

            def mm1(b):
                # energyT[h,s] block-row mt: sum_kt w2tk[kt]^T @ encT[kt]
                pss = [None] * MT
                eTs = [None] * MT
                for half in range(2):
                    cs0 = half * NH
                    for mt in range(MT):
                        if half == 0:
                            eTs[mt] = epool.tile(
                                [128, S], F32, tag="eT", name=f"eT_{b}_{mt}"
                            )
                    for kt in range(KT):
                        wt = w2tk[kt // 2]
                        wo = (kt % 2) * H
                        et = encT[b][kt // 2]
                        eo = (kt % 2) * S + cs0
                        for mt in range(MT):
                            if kt == 0:
                                pss[mt] = ps.tile(
                                    [128, NH], F32, tag="ps", name=f"ps_{b}_{half}_{mt}"
                                )
                            nc.tensor.matmul(
                                pss[mt][:],
                                wt[:, wo + mt * 128 : wo + (mt + 1) * 128],
                                et[:, eo : eo + NH],
                                start=(kt == 0),
                                stop=(kt == KT - 1),
                            )
                            if kt == KT - 1:
                                # evacuate: eT = tanh(psum + htE[:,mt,b])
                                nc.scalar.activation(
                                    eTs[mt][:, cs0 : cs0 + NH],
                                    pss[mt][:],
                                    AF.Tanh,
                                    bias=htE[:, mt * BPC + b : mt * BPC + b + 1],
                                )
                return eTs

            def softmax(b, eTs):
                sums = spool.tile([128, MT], F32, tag="sums")
                rec = spool.tile([128, MT], F32, tag="rec")
                ats = []
                for mt in range(MT):
                    ex = xpool.tile([128, S], BF, tag="ex")
                    nc.scalar.activation(
                        ex[:], eTs[mt][:], AF.Exp, accum_out=sums[:, mt : mt + 1]
                    )
                    nc.vector.reciprocal(rec[:, mt : mt + 1], sums[:, mt : mt + 1])
                    at = apool.tile([128, S], BF, tag="at")
                    nc.vector.tensor_scalar_mul(at[:], ex[:], rec[:, mt : mt + 1])
                    nc.gpsimd.dma_start(
                        out=attT_d[b, mt * 128 : (mt + 1) * 128, :], in_=at[:]
                    )
                    ats.append(at)
                return ats

            def mm2(b, ats):
                for mt2 in range(MT):
                    p0 = ps.tile([128, NH], F32, tag="ps", name=f"p0_{b}_{mt2}")
                    p1 = ps.tile([128, NH], F32, tag="ps", name=f"p1_{b}_{mt2}")
                    for kt in range(KT):
                        lhs = ats[kt][:, mt2 * 128 : (mt2 + 1) * 128]
                        en = enc[b][kt // 4]
                        eo = (kt % 4) * H
                        nc.tensor.matmul(
                            p0[:], lhs, en[:, eo : eo + NH],
                            start=(kt == 0), stop=(kt == KT - 1),
                        )
                        nc.tensor.matmul(
                            p1[:], lhs, en[:, eo + NH : eo + H],
                            start=(kt == 0), stop=(kt == KT - 1),
                        )
                    stg = cstg.tile([128, H], BF, tag="cstg")
                    nc.scalar.copy(out=stg[:, :NH], in_=p0[:])
                    nc.vector.tensor_copy(stg[:, NH:], p1[:])
                    nc.sync.dma_start(
                        out=ctx_d[b, mt2 * 128 : (mt2 + 1) * 128, :], in_=stg[:]
                    )

            eT0 = mm1(0)
            a0 = softmax(0, eT0)
            eT1 = mm1(1)
            mm2(0, a0)
            a1 = softmax(1, eT1)
            mm2(1, a1)

    _split_sync_waits(nc, 1)
    return nc


_NC_CACHE = {}


def _get_nc():
    if "nc" not in _NC_CACHE:
        _NC_CACHE["nc"] = build()
    return _NC_CACHE["nc"]


def _pack(m):
    # [1024, D] -> [128, 8*D] with 128-row tile kt at columns [kt*D,(kt+1)*D)
    d = m.shape[1]
    return np.ascontiguousarray(
        m.reshape(KT, 128, d).transpose(1, 0, 2).reshape(128, KT * d)
    )


def _make_in_maps(ht, enc, W_attn, b_attn):
    import ml_dtypes

    bf = ml_dtypes.bfloat16
    ht = np.asarray(ht, np.float32)
    enc = np.asarray(enc, np.float32)
    W = np.asarray(W_attn, np.float32)
    ba = np.asarray(b_attn, np.float32)

    # w2tk[p, kt*H + mt*128 + j] = W2T[kt*128+p, mt*128+j] (kt-major packing)
    w2tk_p = _pack(W[:, H:].T.copy()).astype(bf)
    # htE_full[b, h] = ht @ W1.T + b_attn  (computed on host, tiny)
    htE_full = (ht @ W[:, :H].T + ba).astype(np.float32)  # [B, H]

    in_maps = []
    for c in range(NCORES):
        bs = slice(BPC * c, BPC * (c + 1))
        enc_c = enc[bs]
        enc_p = np.stack([_pack(enc_c[i]) for i in range(BPC)]).astype(bf)
        encT_p = np.stack([_pack(enc_c[i].T.copy()) for i in range(BPC)]).astype(bf)
        # htE_col[p, mt*BPC + i] = htE_full[bs][i, mt*128 + p]
        htE_c = np.ascontiguousarray(
            htE_full[bs].reshape(BPC, MT, 128).transpose(2, 1, 0).reshape(128, MT * BPC)
        )
        in_maps.append(
            {"enc": enc_p, "encT": encT_p, "w2tk": w2tk_p, "htE": htE_c}
        )
    return in_maps


def _run(in_maps, trace=False):
    res = run_bass_kernel_spmd(
        _get_nc(), in_maps, core_ids=list(range(NCORES)), trace=trace
    )
    ctx = np.concatenate(
        [r["ctx"].astype(np.float32) for r in res.results], axis=0
    )
    att = np.concatenate(
        [r["attT"].transpose(0, 2, 1).astype(np.float32) for r in res.results],
        axis=0,
    )
    return (ctx, att), res


def kernel(ht, encoder_out, W_attn, b_attn, W_v=None, **_unused):
    out, _ = _run(_make_in_maps(ht, encoder_out, W_attn, b_attn), trace=False)
    return out


def kernel_traced(ht, encoder_out, W_attn, b_attn, W_v=None, **_unused):
    """Like kernel() but also returns the BassKernelResults with profile."""
    out, res = _run(_make_in_maps(ht, encoder_out, W_attn, b_attn), trace=True)
    return out, res


# revision 8
# speedup vs baseline: 1.6058x; 1.0271x over previous
"""Trainium2 Bass kernel for nn_AttentionNetwork (B=16, S=H=1024).

reference:
    energy  = tanh(concat([ht bcast, enc], -1) @ W_attn.T + b_attn)   [B,S,H]
    att     = softmax(energy, axis=1)  (over the seq axis)
    context = einsum('bsk,bkh->bsh', att, enc)
    returns (context, att)   (the W_v projection output is dead code)

Strategy (v4):
  - Data-parallel over batch: 2 batches per NeuronCore x 8 cores (SPMD).
  - htE = ht @ W1.T + b_attn computed on HOST (tiny GEMM), shipped as an
    8KB f32 tensor; device only does the two big GEMMs per batch.
  - mm1 computes energy TRANSPOSED (energyT[h,s]) so softmax over s is a
    free-dim reduction. kt-OUTER loop order over all 8 row-blocks (8 PSUM
    banks); encT is packed HALF-MAJOR on the host so the s-half-0 pass
    only needs 1MiB of encT + 2MiB of w2tk up front; w2tk and encT stream
    on different DMA queues in parallel. PE warm-up dummies are sized to
    abut the first real matmul so HAM never re-throttles.
  - softmax: tanh(+bias) PSUM->SBUF f32 on ACT, exp with accum_out sums,
    reciprocal + at = ex*rec (bf16) on DVE (GpSimd tensor ops are slow
    and also wedge DVE's fast mode -- keep GpSimd to DMA triggers only).
  - att is written to DRAM TRANSPOSED (attT[h,s], bf16) straight from the
    at tiles -- no PE transposes; the host transposes + upcasts.
  - mm2: ctx = matmul(lhsT=at, rhs=enc) natural [s,h]; PSUM evacuated as
    bf16 via scalar/vector copy halves; DMA out bf16 per half, host
    upcasts.
  - Engine queues: scalar = activations/copies ONLY (DMA triggers would
    head-of-line block the tanh chain); sync = encT in + ctx out;
    gpsimd = htE/w2tk/enc in + attT out; vector = recip/at-mult/ctx-copy.
  - PE program order: warm dummies, mm1(b0), mm1(b1), mm2(b0), mm2(b1)
    with no PE gaps; everything else rides in the matmul shadow.
"""

import sys
import numpy as np

sys.path.insert(0, "/opt/trn_rl_repo")

import concourse.bass as bass
import concourse.mybir as mybir
import concourse.tile as tile
from concourse.bass_utils import run_bass_kernel_spmd
from concourse.masks import make_identity

F32 = mybir.dt.float32
BF = mybir.dt.bfloat16
AF = mybir.ActivationFunctionType

B, S, H = 16, 1024, 1024
NCORES = 8
BPC = B // NCORES  # batches per core
KT = 8             # 128-row contraction tiles
MT = 8             # output partition tiles
NH = 512           # matmul free-dim chunk (one PSUM bank fp32)
WARM_MM = 44       # [128,512] dummy matmuls bridging the DMA head


def _split_sync_waits(nc, maxw=1):
    """This walrus rejects instructions with more than one sync wait.
    Move excess on_wait entries onto InstNoOp on the same engine queue
    (executed in order ahead of the real instruction)."""
    ctr = 0
    for fn in nc.m.functions:
        for blk in fn.blocks:
            new = []
            for inst in blk.instructions:
                si = inst.sync_info
                if si is not None and si.on_wait and len(si.on_wait) > maxw:
                    waits = list(si.on_wait)
                    extra, keep = waits[:-maxw], waits[-maxw:]
                    for i in range(0, len(extra), maxw):
                        ctr += 1
                        nop = mybir.InstNoOp(
                            name=f"I-ws-{ctr}",
                            engine=inst.engine,
                            sync_info=mybir.SyncInfo(
                                on_wait=extra[i : i + maxw], on_update=[]
                            ),
                        )
                        nc.register_instruction(nop)
                        new.append(nop)
                    inst.sync_info = mybir.SyncInfo(
                        on_wait=keep, on_update=list(si.on_update)
                    )
                new.append(inst)
            blk.instructions[:] = new
    return ctr


def build():
    nc = bass.Bass()
    # encTh[b, half, p, kt*NH + j] = enc[b][half*NH + j, kt*128 + p]
    encTh_d = nc.declare_dram_parameter(
        "encTh", [BPC, 2, 128, KT * NH], BF, isOutput=False
    )
    enc_d = nc.declare_dram_parameter("enc", [BPC, 128, KT * H], BF, isOutput=False)
    w2tk_d = nc.declare_dram_parameter("w2tk", [128, KT * H], BF, isOutput=False)
    htE_d = nc.declare_dram_parameter("htE", [128, MT * BPC], F32, isOutput=False)
    ctx_d = nc.declare_dram_parameter("ctx", [BPC, S, H], BF, isOutput=True)
    attT_d = nc.declare_dram_parameter("attT", [BPC, H, S], BF, isOutput=True)

    NQ = KT // 4  # 4-kt quads per half

    with tile.TileContext(nc) as tc:
        with (
            tc.tile_pool(name="wpool", bufs=1) as wpool,
            tc.tile_pool(name="w2pool", bufs=KT // 2) as w2pool,      # w2tk 2-kt
            tc.tile_pool(name="etpool", bufs=2 * 2 * NQ) as etpool,   # encT quads
            tc.tile_pool(name="enpool", bufs=2 * KT // 4) as enpool,  # enc 4-kt
            tc.tile_pool(name="epool", bufs=MT) as epool,             # eT f32
            tc.tile_pool(name="xpool", bufs=3) as xpool,              # ex bf16
            tc.tile_pool(name="apool", bufs=2 * KT) as apool,         # at bf16
            tc.tile_pool(name="spool", bufs=4) as spool,              # sums/rec
            tc.tile_pool(name="cstg", bufs=4) as cstg,                # ctx staging
            tc.tile_pool(name="ps", bufs=8, space="PSUM") as ps,      # all 8 banks
        ):
            # --- ACT table warm first: nothing sits in front of it on the
            # scalar queue, so the 1.3us spline-table load happens at ~4us.
            warma = wpool.tile([128, 1], F32)
            nc.vector.memset(warma[:], 0.5)
            nc.scalar.activation(warma[:], warma[:], AF.Exp)

            # --- head DMAs, first-use order, two parallel bulk queues:
            # sync queue:   encTh(b0,h0), encTh(b0,h1), encTh(b1,*); ctx out.
            # gpsimd queue: htE (tiny), w2tk, enc b0/b1; attT out.
            encT = [[[None] * NQ for _ in range(2)] for _ in range(BPC)]
            enc = [[None] * (KT // 4) for _ in range(BPC)]
            w2tk = [None] * (KT // 2)
            htE = wpool.tile([128, MT * BPC], F32)
            nc.gpsimd.dma_start(out=htE[:], in_=htE_d[:])
            for b in range(BPC):
                for half in range(2):
                    for q in range(NQ):
                        et = etpool.tile(
                            [128, 4 * NH], BF, tag="encT", name=f"encT_{b}_{half}_{q}"
                        )
                        nc.sync.dma_start(
                            out=et[:],
                            in_=encTh_d[b, half, :, q * 4 * NH : (q + 1) * 4 * NH],
                        )
                        encT[b][half][q] = et
                if b == 0:
                    for kp in range(KT // 2):
                        wt = w2pool.tile(
                            [128, 2 * H], BF, tag="w2tk", name=f"w2tk_{kp}"
                        )
                        nc.gpsimd.dma_start(
                            out=wt[:], in_=w2tk_d[:, kp * 2 * H : (kp + 1) * 2 * H]
                        )
                        w2tk[kp] = wt
            for b in range(BPC):
                for kq in range(KT // 4):
                    e = enpool.tile([128, 4 * H], BF, tag="enc", name=f"enc_{b}_{kq}")
                    nc.gpsimd.dma_start(
                        out=e[:], in_=enc_d[b, :, kq * 4 * H : (kq + 1) * 4 * H]
                    )
                    enc[b][kq] = e

            # --- PE warmup: keep HAM warm while the first chunks stream in.
            dummy = wpool.tile([128, NH], BF)
            nc.vector.memset(dummy[:], 0.0)
            ident = wpool.tile([128, 128], BF)
            make_identity(nc, ident[:])
            warmp = ps.tile([128, NH], F32, tag="ps")
            for i in range(WARM_MM):
                nc.tensor.matmul(
                    warmp[:], ident[:], dummy[:], start=(i == 0), stop=(i == WARM_MM - 1)
                )

            def mm1(b):
                # energyT[h,s] block-row mt: sum_kt w2tk[kt]^T @ encT[kt]
                pss = [None] * MT
                eTs = [None] * MT
                for half in range(2):
                    cs0 = half * NH
                    for mt in range(MT):
                        if half == 0:
                            eTs[mt] = epool.tile(
                                [128, S], F32, tag="eT", name=f"eT_{b}_{mt}"
                            )
                    for kt in range(KT):
                        wt = w2tk[kt // 2]
                        wo = (kt % 2) * H
                        et = encT[b][half][kt // 4]
                        eo = (kt % 4) * NH
                        for mt in range(MT):
                            if kt == 0:
                                pss[mt] = ps.tile(
                                    [128, NH], F32, tag="ps", name=f"ps_{b}_{half}_{mt}"
                                )
                            nc.tensor.matmul(
                                pss[mt][:],
                                wt[:, wo + mt * 128 : wo + (mt + 1) * 128],
                                et[:, eo : eo + NH],
                                start=(kt == 0),
                                stop=(kt == KT - 1),
                            )
                            if kt == KT - 1:
                                # evacuate: eT = tanh(psum + htE[:,mt,b])
                                nc.scalar.activation(
                                    eTs[mt][:, cs0 : cs0 + NH],
                                    pss[mt][:],
                                    AF.Tanh,
                                    bias=htE[:, mt * BPC + b : mt * BPC + b + 1],
                                )
                return eTs

            def softmax(b, eTs):
                sums = spool.tile([128, MT], F32, tag="sums")
                rec = spool.tile([128, MT], F32, tag="rec")
                ats = []
                for mt in range(MT):
                    ex = xpool.tile([128, S], BF, tag="ex")
                    nc.scalar.activation(
                        ex[:], eTs[mt][:], AF.Exp, accum_out=sums[:, mt : mt + 1]
                    )
                    nc.vector.reciprocal(rec[:, mt : mt + 1], sums[:, mt : mt + 1])
                    at = apool.tile([128, S], BF, tag="at")
                    nc.vector.tensor_scalar_mul(at[:], ex[:], rec[:, mt : mt + 1])
                    nc.gpsimd.dma_start(
                        out=attT_d[b, mt * 128 : (mt + 1) * 128, :], in_=at[:]
                    )
                    ats.append(at)
                return ats

            def mm2(b, ats):
                for mt2 in range(MT):
                    p0 = ps.tile([128, NH], F32, tag="ps", name=f"p0_{b}_{mt2}")
                    p1 = ps.tile([128, NH], F32, tag="ps", name=f"p1_{b}_{mt2}")
                    for kt in range(KT):
                        lhs = ats[kt][:, mt2 * 128 : (mt2 + 1) * 128]
                        en = enc[b][kt // 4]
                        eo = (kt % 4) * H
                        nc.tensor.matmul(
                            p0[:], lhs, en[:, eo : eo + NH],
                            start=(kt == 0), stop=(kt == KT - 1),
                        )
                        nc.tensor.matmul(
                            p1[:], lhs, en[:, eo + NH : eo + H],
                            start=(kt == 0), stop=(kt == KT - 1),
                        )
                    stg = cstg.tile([128, H], BF, tag="cstg")
                    nc.scalar.copy(out=stg[:, :NH], in_=p0[:])
                    nc.sync.dma_start(
                        out=ctx_d[b, mt2 * 128 : (mt2 + 1) * 128, :NH],
                        in_=stg[:, :NH],
                    )
                    nc.vector.tensor_copy(stg[:, NH:], p1[:])
                    nc.sync.dma_start(
                        out=ctx_d[b, mt2 * 128 : (mt2 + 1) * 128, NH:],
                        in_=stg[:, NH:],
                    )

            eT0 = mm1(0)
            a0 = softmax(0, eT0)
            eT1 = mm1(1)
            mm2(0, a0)
            a1 = softmax(1, eT1)
            mm2(1, a1)

    _split_sync_waits(nc, 1)
    return nc


_NC_CACHE = {}


def _get_nc():
    if "nc" not in _NC_CACHE:
        _NC_CACHE["nc"] = build()
    return _NC_CACHE["nc"]


def _pack(m):
    # [1024, D] -> [128, 8*D] with 128-row tile kt at columns [kt*D,(kt+1)*D)
    d = m.shape[1]
    return np.ascontiguousarray(
        m.reshape(KT, 128, d).transpose(1, 0, 2).reshape(128, KT * d)
    )


def _make_in_maps(ht, enc, W_attn, b_attn):
    import ml_dtypes

    bf = ml_dtypes.bfloat16
    ht = np.asarray(ht, np.float32)
    enc = np.asarray(enc, np.float32)
    W = np.asarray(W_attn, np.float32)
    ba = np.asarray(b_attn, np.float32)

    # w2tk[p, kt*H + mt*128 + j] = W2T[kt*128+p, mt*128+j] (kt-major packing)
    w2tk_p = _pack(W[:, H:].T.copy()).astype(bf)
    # htE_full[b, h] = ht @ W1.T + b_attn  (computed on host, tiny)
    htE_full = (ht @ W[:, :H].T + ba).astype(np.float32)  # [B, H]

    in_maps = []
    for c in range(NCORES):
        bs = slice(BPC * c, BPC * (c + 1))
        enc_c = enc[bs]
        enc_p = np.stack([_pack(enc_c[i]) for i in range(BPC)]).astype(bf)
        # encTh[b, half, p, kt*NH + j] = enc_c[b][half*NH + j, kt*128 + p]
        encTh_p = np.ascontiguousarray(
            enc_c.reshape(BPC, 2, NH, KT, 128).transpose(0, 1, 4, 3, 2)
        ).reshape(BPC, 2, 128, KT * NH).astype(bf)
        # htE_col[p, mt*BPC + i] = htE_full[bs][i, mt*128 + p]
        htE_c = np.ascontiguousarray(
            htE_full[bs].reshape(BPC, MT, 128).transpose(2, 1, 0).reshape(128, MT * BPC)
        )
        in_maps.append(
            {"enc": enc_p, "encTh": encTh_p, "w2tk": w2tk_p, "htE": htE_c}
        )
    return in_maps


def _run(in_maps, trace=False):
    res = run_bass_kernel_spmd(
        _get_nc(), in_maps, core_ids=list(range(NCORES)), trace=trace
    )
    ctx = np.concatenate(
        [r["ctx"].astype(np.float32) for r in res.results], axis=0
    )
    att = np.concatenate(
        [r["attT"].transpose(0, 2, 1).astype(np.float32) for r in res.results],
        axis=0,
    )
    return (ctx, att), res


def kernel(ht, encoder_out, W_attn, b_attn, W_v=None, **_unused):
    out, _ = _run(_make_in_maps(ht, encoder_out, W_attn, b_attn), trace=False)
    return out


def kernel_traced(ht, encoder_out, W_attn, b_attn, W_v=None, **_unused):
    """Like kernel() but also returns the BassKernelResults with profile."""
    out, res = _run(_make_in_maps(ht, encoder_out, W_attn, b_attn), trace=True)
    return out, res
